# revision 2
# baseline (speedup 1.0000x reference)
"""CG-SENSE (MRI) solver on 8 Trainium2 NeuronCores.

Problem: per-sample complex CG solve of (AH A + lam I) x = AH(y) + lam x0,
A = mask * FFT2(smaps * p) per coil, 10 fixed iterations (the reference's
convergence freeze at tol=1e-10 never triggers for this data regime:
min rTr over the trajectory is ~1e-2).

Sharding: data-parallel over batch B=8 -> one sample per NeuronCore.
Per core, the 2D FFTs are realized as DFT matmuls on the tensor engine in
float32r (TF32-like, 1 cyc/row), corner turns via PE transpose-mode, complex
pointwise ops on GPSIMD/DVE, PSUM->SBUF moves on the scalar engine, CG dot
products via scalar_tensor_tensor accum + an all-ones fp32 matmul for the
cross-partition reduce-and-broadcast.

SBUF layout: every 384x384 matrix is [128 partitions, 3, 384]:
  tile[p, j, w] = M[j*128 + p, w]; complex = separate re/im planes.
One complex DFT matmul = 3 m-blocks x (4 terms x 3 K-chunks) of
[K=128, M=128, N=384] matmuls PSUM-accumulated (re: Fre@Ur + (-Fim)@Ui etc).
"""

import numpy as np

import concourse.bacc as bacc
import concourse.mybir as mybir
from concourse.tile import TileContext
from concourse.masks import make_identity

F32 = mybir.dt.float32
F32R = mybir.dt.float32r
MULT = mybir.AluOpType.mult
ADD = mybir.AluOpType.add
SUB = mybir.AluOpType.subtract

_B, _C, _N = 8, 16, 384
_ITERS = 10


def _build_nc():
    nc = bacc.Bacc("TRN2", target_bir_lowering=False, debug=False)
    C = _C

    def din(name, shape):
        return nc.dram_tensor(name, shape, F32, kind="ExternalInput")

    s_re = din("s_re", [C, 384, 384])
    s_im = din("s_im", [C, 384, 384])
    y_re = din("y_re", [C, 384, 384])
    y_im = din("y_im", [C, 384, 384])
    x_re = din("x_re", [384, 384])
    x_im = din("x_im", [384, 384])
    m2_d = din("m2", [384, 384])
    m2t_d = din("m2t", [384, 384])
    fre_d = din("fre", [384, 384])
    fim_d = din("fim", [384, 384])
    fimn_d = din("fimn", [384, 384])
    lam_d = din("lam_b", [128, 1])
    o_re = nc.dram_tensor("o_re", [384, 384], F32, kind="ExternalOutput")
    o_im = nc.dram_tensor("o_im", [384, 384], F32, kind="ExternalOutput")

    def r3(d):
        return d.rearrange("(j p) w -> p j w", p=128)

    with TileContext(nc) as tc:
        with tc.tile_pool(name="const", bufs=1) as cpool, \
             tc.tile_pool(name="state", bufs=1) as spool, \
             tc.tile_pool(name="coil", bufs=2) as kpool, \
             tc.tile_pool(name="psA", bufs=3, space="PSUM") as psA, \
             tc.tile_pool(name="psB", bufs=2, space="PSUM") as psB:

            # ---- constants ----
            cf = cpool.tile([128, 3, 384], F32, tag="cf", name="cf")
            fre = cpool.tile([128, 3, 384], F32R, tag="c3", name="c3")
            fim = cpool.tile([128, 3, 384], F32R, tag="c4", name="c4")
            fimn = cpool.tile([128, 3, 384], F32R, tag="c5", name="c5")
            nc.sync.dma_start(cf[:, :, :], r3(fre_d))
            nc.vector.tensor_copy(fre[:, :, :], cf[:, :, :])
            nc.sync.dma_start(cf[:, :, :], r3(fim_d))
            nc.vector.tensor_copy(fim[:, :, :], cf[:, :, :])
            nc.sync.dma_start(cf[:, :, :], r3(fimn_d))
            nc.vector.tensor_copy(fimn[:, :, :], cf[:, :, :])
            m2 = cpool.tile([128, 3, 384], F32, tag="c6", name="c6")
            m2t = cpool.tile([128, 3, 384], F32, tag="c7", name="c7")
            nc.sync.dma_start(m2[:, :, :], r3(m2_d))
            nc.sync.dma_start(m2t[:, :, :], r3(m2t_d))
            ident = cpool.tile([128, 128], F32, tag="c8", name="c8")
            make_identity(nc, ident[:, :])
            ones = cpool.tile([128, 128], F32, tag="c9", name="c9")
            nc.vector.memset(ones[:, :], 1.0)
            lam_t = cpool.tile([128, 1], F32, tag="cl", name="cl")
            nc.sync.dma_start(lam_t[:, :], lam_d[:, :])

            # ---- CG state ----
            def cplx(tag):
                return (spool.tile([128, 3, 384], F32, tag=tag + "r", name=tag + "r"),
                        spool.tile([128, 3, 384], F32, tag=tag + "i", name=tag + "i"))
            xs = cplx("x")
            r = cplx("r")
            p = cplx("p")
            acc = cplx("a")
            sc = {}
            for nm in ("rtr", "rtrn", "pqr", "pqi", "t0", "t1",
                       "alr", "ali", "nalr", "nali", "bet"):
                sc[nm] = spool.tile([128, 1], F32, tag="s" + nm, name="s" + nm)
            junk = spool.tile([128, 3, 384], F32, tag="junk", name="junk")

            # ---- helpers ----
            def cmm(ps_re, ps_im, lre, lpos, lneg, rhs, m):
                rr, ri = rhs
                for j in range(3):
                    sl = lambda t: t[:, j, m * 128:(m + 1) * 128]
                    nc.tensor.matmul(ps_re[:, :], sl(lre), rr[:, j, :],
                                     start=(j == 0), stop=False)
                    nc.tensor.matmul(ps_im[:, :], sl(lre), ri[:, j, :],
                                     start=(j == 0), stop=False)
                    nc.tensor.matmul(ps_re[:, :], sl(lneg), ri[:, j, :],
                                     start=False, stop=(j == 2))
                    nc.tensor.matmul(ps_im[:, :], sl(lpos), rr[:, j, :],
                                     start=False, stop=(j == 2))

            def mmstage(dst, rhs, lre, lpos, lneg, mask_t=None):
                dr, di = dst
                for m in range(3):
                    ps_re = psA.tile([128, 384], F32, tag="psr", name="psr")
                    ps_im = psA.tile([128, 384], F32, tag="psi", name="psi")
                    cmm(ps_re, ps_im, lre, lpos, lneg, rhs, m)
                    if mask_t is None:
                        nc.scalar.copy(dr[:, m, :], ps_re[:, :])
                        nc.scalar.copy(di[:, m, :], ps_im[:, :])
                    else:
                        nc.vector.tensor_tensor(dr[:, m, :], ps_re[:, :],
                                                mask_t[:, m, :], op=MULT)
                        nc.vector.tensor_tensor(di[:, m, :], ps_im[:, :],
                                                mask_t[:, m, :], op=MULT)

            def ptrans(dst, src):
                for dr, sr in zip(dst, src):
                    for mp in range(3):
                        ps = psB.tile([128, 384], F32, tag="pst", name="pst")
                        for jp in range(3):
                            nc.tensor.matmul(
                                ps[:, jp * 128:(jp + 1) * 128],
                                sr[:, jp, mp * 128:(mp + 1) * 128],
                                ident[:, :], is_transpose=True)
                        nc.scalar.copy(dr[:, mp, :], ps[:, :])

            def coil_tiles():
                return dict(
                    s=(kpool.tile([128, 3, 384], F32, tag="sre", name="sre"),
                       kpool.tile([128, 3, 384], F32, tag="sim", name="sim")),
                    u2=(kpool.tile([128, 3, 384], F32R, tag="u2r", name="u2r"),
                        kpool.tile([128, 3, 384], F32R, tag="u2i", name="u2i")),
                    fr=(kpool.tile([128, 3, 384], F32, tag="frr", name="frr"),
                        kpool.tile([128, 3, 384], F32, tag="fri", name="fri")),
                    ft=(kpool.tile([128, 3, 384], F32R, tag="ftr", name="ftr"),
                        kpool.tile([128, 3, 384], F32R, tag="fti", name="fti")),
                    w=(kpool.tile([128, 3, 384], F32R, tag="wr", name="wr"),
                       kpool.tile([128, 3, 384], F32R, tag="wi", name="wi")),
                    t1=kpool.tile([128, 3, 384], F32, tag="t1w", name="t1w"),
                    t2=kpool.tile([128, 3, 384], F32, tag="t2w", name="t2w"),
                )

            def load_coil(c, dst, src_re, src_im):
                nc.sync.dma_start(dst[0][:, :, :],
                                  src_re[c].rearrange("(j p) w -> p j w", p=128))
                nc.sync.dma_start(dst[1][:, :, :],
                                  src_im[c].rearrange("(j p) w -> p j w", p=128))

            def combine(accvec, s_t, z_ps_re, z_ps_im, m, tm):
                ar_, ai_ = accvec
                sr_, si_ = s_t
                nc.vector.tensor_tensor(tm[:, :], z_ps_re[:, :], sr_[:, m, :], op=MULT)
                nc.vector.tensor_tensor(ar_[:, m, :], ar_[:, m, :], tm[:, :], op=ADD)
                nc.vector.tensor_tensor(tm[:, :], z_ps_im[:, :], si_[:, m, :], op=MULT)
                nc.vector.tensor_tensor(ar_[:, m, :], ar_[:, m, :], tm[:, :], op=ADD)
                nc.vector.tensor_tensor(tm[:, :], z_ps_im[:, :], sr_[:, m, :], op=MULT)
                nc.vector.tensor_tensor(ai_[:, m, :], ai_[:, m, :], tm[:, :], op=ADD)
                nc.vector.tensor_tensor(tm[:, :], z_ps_re[:, :], si_[:, m, :], op=MULT)
                nc.vector.tensor_tensor(ai_[:, m, :], ai_[:, m, :], tm[:, :], op=SUB)

            def coil_chain(ct, pvec, accvec):
                pr, pi = pvec
                sr_, si_ = ct["s"]
                u = ct["u2"]
                t1_, t2_ = ct["t1"], ct["t2"]
                nc.gpsimd.tensor_tensor(t1_[:, :, :], sr_[:, :, :], pr[:, :, :], op=MULT)
                nc.gpsimd.tensor_tensor(t2_[:, :, :], si_[:, :, :], pi[:, :, :], op=MULT)
                nc.gpsimd.tensor_tensor(u[0][:, :, :], t1_[:, :, :], t2_[:, :, :], op=SUB)
                nc.gpsimd.tensor_tensor(t1_[:, :, :], sr_[:, :, :], pi[:, :, :], op=MULT)
                nc.gpsimd.tensor_tensor(t2_[:, :, :], si_[:, :, :], pr[:, :, :], op=MULT)
                nc.gpsimd.tensor_tensor(u[1][:, :, :], t1_[:, :, :], t2_[:, :, :], op=ADD)

                v = ct["fr"]
                mmstage(v, u, fre, fim, fimn)                # V = F@U
                vt = ct["ft"]
                ptrans(vt, v)                                # V^T
                w = ct["w"]
                mmstage(w, vt, fre, fim, fimn, mask_t=m2t)   # W^T = m2t*(F@V^T)
                rp = ct["fr"]
                mmstage(rp, w, fre, fimn, fim)               # R' = Fi@W^T
                rpt = ct["ft"]
                ptrans(rpt, rp)                              # R'^T
                for m in range(3):
                    ps_re = psA.tile([128, 384], F32, tag="psr", name="psr")
                    ps_im = psA.tile([128, 384], F32, tag="psi", name="psi")
                    cmm(ps_re, ps_im, fre, fimn, fim, rpt, m)   # Z = Fi@R'^T
                    tm = kpool.tile([128, 384], F32, tag="tm", name="tm")
                    combine(accvec, ct["s"], ps_re, ps_im, m, tm)

            def dot2(dst, a0, a1, b0, b1, op_out):
                nc.vector.scalar_tensor_tensor(
                    junk[:, :, :], a0[:, :, :], 1.0, b0[:, :, :],
                    op0=MULT, op1=MULT, accum_out=sc["t0"][:, :])
                nc.vector.scalar_tensor_tensor(
                    junk[:, :, :], a1[:, :, :], 1.0, b1[:, :, :],
                    op0=MULT, op1=MULT, accum_out=sc["t1"][:, :])
                nc.vector.tensor_tensor(dst[:, :], sc["t0"][:, :], sc["t1"][:, :],
                                        op=op_out)

            def allred(t):
                ps = psB.tile([128, 1], F32, tag="pst", name="psred")
                nc.tensor.matmul(ps[:, :], ones[:, :], t[:, :], start=True, stop=True)
                nc.scalar.copy(t[:, :], ps[:, :])

            # ---- rhs = AH(y) + lam*x  (into r) ----
            for comp in acc + r + xs:
                nc.vector.memset(comp[:, :, :], 0.0)

            for c in range(C):
                ct = coil_tiles()
                load_coil(c, ct["s"], y_re, y_im)
                w = ct["w"]
                nc.vector.tensor_tensor(w[0][:, :, :], ct["s"][0][:, :, :],
                                        m2[:, :, :], op=MULT)
                nc.vector.tensor_tensor(w[1][:, :, :], ct["s"][1][:, :, :],
                                        m2[:, :, :], op=MULT)
                pm = ct["fr"]
                mmstage(pm, w, fre, fimn, fim)         # P = Fi@W  [h, kw]
                pt = ct["ft"]
                ptrans(pt, pm)                         # P^T [kw, h]
                zt = (ct["u2"][0].bitcast(F32), ct["u2"][1].bitcast(F32))
                mmstage((ct["u2"][0], ct["u2"][1]), pt, fre, fimn, fim)  # Z^T
                zf = ct["fr"]
                nc.vector.tensor_copy(zf[0][:, :, :], zt[0][:, :, :])
                nc.vector.tensor_copy(zf[1][:, :, :], zt[1][:, :, :])
                load_coil(c, ct["s"], s_re, s_im)
                for m in range(3):
                    ps_re = psB.tile([128, 384], F32, tag="pst", name="pst")
                    ps_im = psA.tile([128, 384], F32, tag="psr", name="psr")
                    for jp in range(3):
                        nc.tensor.matmul(ps_re[:, jp * 128:(jp + 1) * 128],
                                         zf[0][:, jp, m * 128:(m + 1) * 128],
                                         ident[:, :], is_transpose=True)
                        nc.tensor.matmul(ps_im[:, jp * 128:(jp + 1) * 128],
                                         zf[1][:, jp, m * 128:(m + 1) * 128],
                                         ident[:, :], is_transpose=True)
                    tm = kpool.tile([128, 384], F32, tag="tm", name="tm")
                    combine(r, ct["s"], ps_re, ps_im, m, tm)

            xf = (kpool.tile([128, 3, 384], F32, tag="sre", name="sre"),
                  kpool.tile([128, 3, 384], F32, tag="sim", name="sim"))
            nc.sync.dma_start(xf[0][:, :, :], r3(x_re))
            nc.sync.dma_start(xf[1][:, :, :], r3(x_im))
            nc.vector.scalar_tensor_tensor(r[0][:, :, :], xf[0][:, :, :],
                                           lam_t[:, 0:1], r[0][:, :, :],
                                           op0=MULT, op1=ADD)
            nc.vector.scalar_tensor_tensor(r[1][:, :, :], xf[1][:, :, :],
                                           lam_t[:, 0:1], r[1][:, :, :],
                                           op0=MULT, op1=ADD)
            nc.vector.tensor_copy(p[0][:, :, :], r[0][:, :, :])
            nc.vector.tensor_copy(p[1][:, :, :], r[1][:, :, :])
            dot2(sc["rtr"], r[0], r[1], r[0], r[1], ADD)
            allred(sc["rtr"])

            # ---- CG iterations ----
            def iteration(_=None):
                nc.vector.tensor_scalar(acc[0][:, :, :], p[0][:, :, :],
                                        lam_t[:, 0:1], None, op0=MULT)
                nc.vector.tensor_scalar(acc[1][:, :, :], p[1][:, :, :],
                                        lam_t[:, 0:1], None, op0=MULT)
                for c in range(C):
                    ct = coil_tiles()
                    load_coil(c, ct["s"], s_re, s_im)
                    coil_chain(ct, p, acc)
                # pq = <p, q>
                dot2(sc["pqr"], p[0], p[1], acc[0], acc[1], ADD)
                allred(sc["pqr"])
                nc.vector.scalar_tensor_tensor(
                    junk[:, :, :], p[0][:, :, :], 1.0, acc[1][:, :, :],
                    op0=MULT, op1=MULT, accum_out=sc["t0"][:, :])
                nc.vector.scalar_tensor_tensor(
                    junk[:, :, :], p[1][:, :, :], 1.0, acc[0][:, :, :],
                    op0=MULT, op1=MULT, accum_out=sc["t1"][:, :])
                nc.vector.tensor_tensor(sc["pqi"][:, :], sc["t0"][:, :],
                                        sc["t1"][:, :], op=SUB)
                allred(sc["pqi"])
                # alpha = rtr*(pqr - i pqi)/(pqr^2+pqi^2)
                nc.vector.tensor_tensor(sc["t0"][:, :], sc["pqr"][:, :], sc["pqr"][:, :], op=MULT)
                nc.vector.tensor_tensor(sc["t1"][:, :], sc["pqi"][:, :], sc["pqi"][:, :], op=MULT)
                nc.vector.tensor_tensor(sc["t0"][:, :], sc["t0"][:, :], sc["t1"][:, :], op=ADD)
                nc.vector.reciprocal(sc["t0"][:, :], sc["t0"][:, :])
                nc.vector.tensor_tensor(sc["t1"][:, :], sc["rtr"][:, :], sc["t0"][:, :], op=MULT)
                nc.vector.tensor_tensor(sc["alr"][:, :], sc["pqr"][:, :], sc["t1"][:, :], op=MULT)
                nc.vector.tensor_tensor(sc["nali"][:, :], sc["pqi"][:, :], sc["t1"][:, :], op=MULT)
                nc.vector.tensor_scalar_mul(sc["ali"][:, :], sc["nali"][:, :], -1.0)
                nc.vector.tensor_scalar_mul(sc["nalr"][:, :], sc["alr"][:, :], -1.0)
                # x += alpha*p ; r -= alpha*q
                nc.vector.scalar_tensor_tensor(xs[0][:, :, :], p[0][:, :, :], sc["alr"][:, 0:1],
                                               xs[0][:, :, :], op0=MULT, op1=ADD)
                nc.vector.scalar_tensor_tensor(xs[0][:, :, :], p[1][:, :, :], sc["nali"][:, 0:1],
                                               xs[0][:, :, :], op0=MULT, op1=ADD)
                nc.vector.scalar_tensor_tensor(xs[1][:, :, :], p[1][:, :, :], sc["alr"][:, 0:1],
                                               xs[1][:, :, :], op0=MULT, op1=ADD)
                nc.vector.scalar_tensor_tensor(xs[1][:, :, :], p[0][:, :, :], sc["ali"][:, 0:1],
                                               xs[1][:, :, :], op0=MULT, op1=ADD)
                nc.vector.scalar_tensor_tensor(r[0][:, :, :], acc[0][:, :, :], sc["nalr"][:, 0:1],
                                               r[0][:, :, :], op0=MULT, op1=ADD)
                nc.vector.scalar_tensor_tensor(r[0][:, :, :], acc[1][:, :, :], sc["ali"][:, 0:1],
                                               r[0][:, :, :], op0=MULT, op1=ADD)
                nc.vector.scalar_tensor_tensor(r[1][:, :, :], acc[1][:, :, :], sc["nalr"][:, 0:1],
                                               r[1][:, :, :], op0=MULT, op1=ADD)
                nc.vector.scalar_tensor_tensor(r[1][:, :, :], acc[0][:, :, :], sc["nali"][:, 0:1],
                                               r[1][:, :, :], op0=MULT, op1=ADD)
                # rtrn, beta, p update
                dot2(sc["rtrn"], r[0], r[1], r[0], r[1], ADD)
                allred(sc["rtrn"])
                nc.vector.reciprocal(sc["t0"][:, :], sc["rtr"][:, :])
                nc.vector.tensor_tensor(sc["bet"][:, :], sc["rtrn"][:, :], sc["t0"][:, :], op=MULT)
                nc.vector.scalar_tensor_tensor(p[0][:, :, :], p[0][:, :, :], sc["bet"][:, 0:1],
                                               r[0][:, :, :], op0=MULT, op1=ADD)
                nc.vector.scalar_tensor_tensor(p[1][:, :, :], p[1][:, :, :], sc["bet"][:, 0:1],
                                               r[1][:, :, :], op0=MULT, op1=ADD)
                nc.vector.tensor_copy(sc["rtr"][:, :], sc["rtrn"][:, :])

            with tc.For_i(0, _ITERS) as _:
                iteration()

            nc.sync.dma_start(o_re.rearrange("(j p) w -> p j w", p=128), xs[0][:, :, :])
            nc.sync.dma_start(o_im.rearrange("(j p) w -> p j w", p=128), xs[1][:, :, :])

    nc.compile()
    return nc


class _Runner:
    """Build-once/run-many wrapper over bass2jax on n_cores axon TRN2 cores."""

    def __init__(self, nc, n_cores):
        import jax
        from jax.sharding import Mesh, PartitionSpec
        from jax.experimental.shard_map import shard_map
        from concourse.bass2jax import (_bass_exec_p, install_neuronx_cc_hook,
                                        partition_id_tensor)
        install_neuronx_cc_hook()
        self.n_cores = n_cores
        partition_name = nc.partition_id_tensor.name if nc.partition_id_tensor else None
        in_names, out_names, out_avals, zero_outs = [], [], [], []
        for alloc in nc.m.functions[0].allocations:
            if not isinstance(alloc, mybir.MemoryLocationSet):
                continue
            name = alloc.memorylocations[0].name
            if alloc.kind == "ExternalInput":
                if name != partition_name:
                    in_names.append(name)
            elif alloc.kind == "ExternalOutput":
                shape = tuple(alloc.tensor_shape)
                dtype = mybir.dt.np(alloc.dtype)
                out_names.append(name)
                out_avals.append(jax.core.ShapedArray(shape, dtype))
                zero_outs.append(np.zeros(shape, dtype))
        self.in_names, self.out_names = in_names, out_names
        self.out_avals, self.zero_outs = out_avals, zero_outs
        n_params, n_outs = len(in_names), len(out_avals)
        all_in = list(in_names) + list(out_names)
        if partition_name is not None:
            all_in.append(partition_name)

        def _body(*args):
            operands = list(args)
            if partition_name is not None:
                operands.append(partition_id_tensor())
            outs = _bass_exec_p.bind(
                *operands, out_avals=tuple(out_avals), in_names=tuple(all_in),
                out_names=tuple(out_names), lowering_input_output_aliases=(),
                sim_require_finite=True, sim_require_nnan=True, nc=nc)
            return tuple(outs)

        donate = tuple(range(n_params, n_params + n_outs))
        devices = jax.devices()[:n_cores]
        mesh = Mesh(np.asarray(devices), ("core",))
        in_specs = (PartitionSpec("core"),) * (n_params + n_outs)
        out_specs = (PartitionSpec("core"),) * len(out_names)
        self._fn = jax.jit(
            shard_map(_body, mesh=mesh, in_specs=in_specs, out_specs=out_specs,
                      check_rep=False),
            donate_argnums=donate, keep_unused=True)
        self.n_params = n_params

    def run(self, in_maps):
        per_core = [[np.asarray(m[n]) for n in self.in_names] for m in in_maps]
        concat_in = [np.concatenate([per_core[c][i] for c in range(self.n_cores)],
                                    axis=0) for i in range(self.n_params)]
        concat_zeros = [np.zeros((self.n_cores * z.shape[0], *z.shape[1:]), z.dtype)
                        for z in self.zero_outs]
        out_arrs = self._fn(*concat_in, *concat_zeros)
        return [
            {n: np.asarray(out_arrs[i]).reshape(self.n_cores,
                                                *self.out_avals[i].shape)[c]
             for i, n in enumerate(self.out_names)}
            for c in range(self.n_cores)
        ]


_RUNNER = None


def _get_runner():
    global _RUNNER
    if _RUNNER is None:
        _RUNNER = _Runner(_build_nc(), _B)
    return _RUNNER


def _make_consts():
    N = _N
    k = np.arange(N)
    ang = -2.0 * np.pi * np.outer(k, k) / N
    fre = (np.cos(ang) / np.sqrt(N)).astype(np.float32)
    fim = (np.sin(ang) / np.sqrt(N)).astype(np.float32)
    return fre, fim, (-fim).astype(np.float32)


def kernel(lambdaa, x, y, smaps, mask, mu):
    lambdaa = np.asarray(lambdaa, np.float32)
    x = np.asarray(x, np.float32)
    y = np.asarray(y, np.float32)
    smaps = np.asarray(smaps, np.float32)
    fre, fim, fimn = _make_consts()
    m = np.asarray(mask).astype(np.float32)
    mt = np.ascontiguousarray(m.T)
    lam_b = np.full((128, 1), float(lambdaa[0]), np.float32)
    runner = _get_runner()
    in_maps = []
    for b in range(_B):
        in_maps.append({
            "s_re": np.ascontiguousarray(smaps[b, :, :, :, 0]),
            "s_im": np.ascontiguousarray(smaps[b, :, :, :, 1]),
            "y_re": np.ascontiguousarray(y[b, :, :, :, 0]),
            "y_im": np.ascontiguousarray(y[b, :, :, :, 1]),
            "x_re": np.ascontiguousarray(x[b, :, :, 0]),
            "x_im": np.ascontiguousarray(x[b, :, :, 1]),
            "m2": m, "m2t": mt,
            "fre": fre, "fim": fim, "fimn": fimn,
            "lam_b": lam_b,
        })
    outs = runner.run(in_maps)
    result = np.empty((_B, _N, _N, 2), np.float32)
    for b in range(_B):
        result[b, :, :, 0] = outs[b]["o_re"]
        result[b, :, :, 1] = outs[b]["o_im"]
    return result


# revision 3
# speedup vs baseline: 1.8095x; 1.8095x over previous
"""CG-SENSE (MRI) solver on 8 Trainium2 NeuronCores.

Problem: per-sample complex CG solve of (AH A + lam I) x = AH(y) + lam x0,
A = mask * FFT2(smaps * p) per coil, 10 fixed iterations (the reference's
convergence freeze at tol=1e-10 never triggers for this data regime:
min rTr over the trajectory is ~1e-2).

Sharding: data-parallel over batch B=8 -> one sample per NeuronCore.
Per core, the 2D FFTs are realized as DFT matmuls on the tensor engine in
float32r (TF32-like, 1 cyc/row), corner turns via PE transpose-mode, complex
pointwise ops on GPSIMD/DVE, PSUM->SBUF moves on the scalar engine, CG dot
products via scalar_tensor_tensor accum + an all-ones fp32 matmul for the
cross-partition reduce-and-broadcast.

SBUF layout: every 384x384 matrix is [128 partitions, 3, 384]:
  tile[p, j, w] = M[j*128 + p, w]; complex = separate re/im planes.
One complex DFT matmul = 3 m-blocks x (4 terms x 3 K-chunks) of
[K=128, M=128, N=384] matmuls PSUM-accumulated (re: Fre@Ur + (-Fim)@Ui etc).
"""

import numpy as np
import ml_dtypes

_BF16NP = ml_dtypes.bfloat16

import concourse.bacc as bacc
import concourse.mybir as mybir
from concourse.tile import TileContext
from concourse.masks import make_identity

F32 = mybir.dt.float32
F32R = mybir.dt.float32r
MULT = mybir.AluOpType.mult
ADD = mybir.AluOpType.add
SUB = mybir.AluOpType.subtract

_B, _C, _N = 8, 16, 384
_ITERS = 10


def _build_nc():
    nc = bacc.Bacc("TRN2", target_bir_lowering=False, debug=False)
    C = _C

    def din(name, shape):
        return nc.dram_tensor(name, shape, F32, kind="ExternalInput")

    BF16 = mybir.dt.bfloat16
    s_re = nc.dram_tensor("s_re", [C, 384, 384], BF16, kind="ExternalInput")
    s_im = nc.dram_tensor("s_im", [C, 384, 384], BF16, kind="ExternalInput")
    y_re = nc.dram_tensor("y_re", [C, 384, 384], BF16, kind="ExternalInput")
    y_im = nc.dram_tensor("y_im", [C, 384, 384], BF16, kind="ExternalInput")
    x_re = din("x_re", [384, 384])
    x_im = din("x_im", [384, 384])
    m2_d = din("m2", [384, 384])
    m2t_d = din("m2t", [384, 384])
    fre_d = din("fre", [384, 384])
    fim_d = din("fim", [384, 384])
    fimn_d = din("fimn", [384, 384])
    lam_d = din("lam_b", [128, 1])
    o_re = nc.dram_tensor("o_re", [384, 384], F32, kind="ExternalOutput")
    o_im = nc.dram_tensor("o_im", [384, 384], F32, kind="ExternalOutput")

    def r3(d):
        return d.rearrange("(j p) w -> p j w", p=128)

    with TileContext(nc) as tc:
        with tc.tile_pool(name="const", bufs=1) as cpool, \
             tc.tile_pool(name="state", bufs=1) as spool, \
             tc.tile_pool(name="coil", bufs=2) as kpool, \
             tc.tile_pool(name="psA", bufs=3, space="PSUM") as psA, \
             tc.tile_pool(name="psB", bufs=2, space="PSUM") as psB:

            # ---- constants ----
            cf = cpool.tile([128, 3, 384], F32, tag="cf", name="cf")
            fre = cpool.tile([128, 3, 384], F32R, tag="c3", name="c3")
            fim = cpool.tile([128, 3, 384], F32R, tag="c4", name="c4")
            fimn = cpool.tile([128, 3, 384], F32R, tag="c5", name="c5")
            nc.sync.dma_start(cf[:, :, :], r3(fre_d))
            nc.vector.tensor_copy(fre[:, :, :], cf[:, :, :])
            nc.sync.dma_start(cf[:, :, :], r3(fim_d))
            nc.vector.tensor_copy(fim[:, :, :], cf[:, :, :])
            nc.sync.dma_start(cf[:, :, :], r3(fimn_d))
            nc.vector.tensor_copy(fimn[:, :, :], cf[:, :, :])
            m2 = cpool.tile([128, 3, 384], F32, tag="c6", name="c6")
            m2t = cpool.tile([128, 3, 384], F32, tag="c7", name="c7")
            nc.sync.dma_start(m2[:, :, :], r3(m2_d))
            nc.sync.dma_start(m2t[:, :, :], r3(m2t_d))
            ident = cpool.tile([128, 128], F32, tag="c8", name="c8")
            make_identity(nc, ident[:, :])
            ones = cpool.tile([128, 128], F32, tag="c9", name="c9")
            nc.vector.memset(ones[:, :], 1.0)
            lam_t = cpool.tile([128, 1], F32, tag="cl", name="cl")
            nc.sync.dma_start(lam_t[:, :], lam_d[:, :])

            # ---- CG state ----
            def cplx(tag):
                return (spool.tile([128, 3, 384], F32, tag=tag + "r", name=tag + "r"),
                        spool.tile([128, 3, 384], F32, tag=tag + "i", name=tag + "i"))
            xs = cplx("x")
            r = cplx("r")
            p = cplx("p")
            acc = cplx("a")
            sc = {}
            for nm in ("rtr", "rtrn", "pqr", "pqi", "t0", "t1",
                       "alr", "ali", "nalr", "nali", "bet"):
                sc[nm] = spool.tile([128, 1], F32, tag="s" + nm, name="s" + nm)
            junk = spool.tile([128, 3, 384], F32, tag="junk", name="junk")

            # ---- helpers ----
            def cmm(ps_re, ps_im, lre, lpos, lneg, rhs, m):
                rr, ri = rhs
                for j in range(3):
                    sl = lambda t: t[:, j, m * 128:(m + 1) * 128]
                    nc.tensor.matmul(ps_re[:, :], sl(lre), rr[:, j, :],
                                     start=(j == 0), stop=False)
                    nc.tensor.matmul(ps_im[:, :], sl(lre), ri[:, j, :],
                                     start=(j == 0), stop=False)
                    nc.tensor.matmul(ps_re[:, :], sl(lneg), ri[:, j, :],
                                     start=False, stop=(j == 2))
                    nc.tensor.matmul(ps_im[:, :], sl(lpos), rr[:, j, :],
                                     start=False, stop=(j == 2))

            def mmstage(dst, rhs, lre, lpos, lneg, mask_t=None):
                dr, di = dst
                for m in range(3):
                    ps_re = psA.tile([128, 384], F32, tag="psr", name="psr")
                    ps_im = psA.tile([128, 384], F32, tag="psi", name="psi")
                    cmm(ps_re, ps_im, lre, lpos, lneg, rhs, m)
                    if mask_t is None:
                        nc.scalar.copy(dr[:, m, :], ps_re[:, :])
                        nc.scalar.copy(di[:, m, :], ps_im[:, :])
                    else:
                        nc.vector.tensor_tensor(dr[:, m, :], ps_re[:, :],
                                                mask_t[:, m, :], op=MULT)
                        nc.vector.tensor_tensor(di[:, m, :], ps_im[:, :],
                                                mask_t[:, m, :], op=MULT)

            def ptrans(dst, src):
                for dr, sr in zip(dst, src):
                    for mp in range(3):
                        ps = psB.tile([128, 384], F32, tag="pst", name="pst")
                        for jp in range(3):
                            nc.tensor.matmul(
                                ps[:, jp * 128:(jp + 1) * 128],
                                sr[:, jp, mp * 128:(mp + 1) * 128],
                                ident[:, :], is_transpose=True)
                        nc.scalar.copy(dr[:, mp, :], ps[:, :])

            def coil_tiles():
                return dict(
                    s=(kpool.tile([128, 3, 384], F32, tag="sre", name="sre"),
                       kpool.tile([128, 3, 384], F32, tag="sim", name="sim")),
                    u2=(kpool.tile([128, 3, 384], F32R, tag="u2r", name="u2r"),
                        kpool.tile([128, 3, 384], F32R, tag="u2i", name="u2i")),
                    fr=(kpool.tile([128, 3, 384], F32, tag="frr", name="frr"),
                        kpool.tile([128, 3, 384], F32, tag="fri", name="fri")),
                    ft=(kpool.tile([128, 3, 384], F32R, tag="ftr", name="ftr"),
                        kpool.tile([128, 3, 384], F32R, tag="fti", name="fti")),
                    w=(kpool.tile([128, 3, 384], F32R, tag="wr", name="wr"),
                       kpool.tile([128, 3, 384], F32R, tag="wi", name="wi")),
                    t1=kpool.tile([128, 3, 384], F32, tag="t1w", name="t1w"),
                    t2=kpool.tile([128, 3, 384], F32, tag="t2w", name="t2w"),
                    sb=(kpool.tile([128, 3, 384], mybir.dt.bfloat16, tag="sbr", name="sbr"),
                        kpool.tile([128, 3, 384], mybir.dt.bfloat16, tag="sbi", name="sbi")),
                )

            def load_coil(c, dst, src_re, src_im, stage):
                nc.sync.dma_start(stage[0][:, :, :],
                                  src_re[c].rearrange("(j p) w -> p j w", p=128))
                nc.sync.dma_start(stage[1][:, :, :],
                                  src_im[c].rearrange("(j p) w -> p j w", p=128))
                nc.vector.tensor_copy(dst[0][:, :, :], stage[0][:, :, :])
                nc.vector.tensor_copy(dst[1][:, :, :], stage[1][:, :, :])

            def combine(accvec, s_t, z_ps_re, z_ps_im, m, tm):
                ar_, ai_ = accvec
                sr_, si_ = s_t
                nc.vector.tensor_tensor(tm[:, :], z_ps_re[:, :], sr_[:, m, :], op=MULT)
                nc.vector.tensor_tensor(ar_[:, m, :], ar_[:, m, :], tm[:, :], op=ADD)
                nc.vector.tensor_tensor(tm[:, :], z_ps_im[:, :], si_[:, m, :], op=MULT)
                nc.vector.tensor_tensor(ar_[:, m, :], ar_[:, m, :], tm[:, :], op=ADD)
                nc.vector.tensor_tensor(tm[:, :], z_ps_im[:, :], sr_[:, m, :], op=MULT)
                nc.vector.tensor_tensor(ai_[:, m, :], ai_[:, m, :], tm[:, :], op=ADD)
                nc.vector.tensor_tensor(tm[:, :], z_ps_re[:, :], si_[:, m, :], op=MULT)
                nc.vector.tensor_tensor(ai_[:, m, :], ai_[:, m, :], tm[:, :], op=SUB)

            def coil_chain(ct, pvec, accvec):
                pr, pi = pvec
                sr_, si_ = ct["s"]
                u = ct["u2"]
                t1_, t2_ = ct["t1"], ct["t2"]
                nc.gpsimd.tensor_tensor(t1_[:, :, :], sr_[:, :, :], pr[:, :, :], op=MULT)
                nc.gpsimd.tensor_tensor(t2_[:, :, :], si_[:, :, :], pi[:, :, :], op=MULT)
                nc.gpsimd.tensor_tensor(u[0][:, :, :], t1_[:, :, :], t2_[:, :, :], op=SUB)
                nc.gpsimd.tensor_tensor(t1_[:, :, :], sr_[:, :, :], pi[:, :, :], op=MULT)
                nc.gpsimd.tensor_tensor(t2_[:, :, :], si_[:, :, :], pr[:, :, :], op=MULT)
                nc.gpsimd.tensor_tensor(u[1][:, :, :], t1_[:, :, :], t2_[:, :, :], op=ADD)

                v = ct["fr"]
                mmstage(v, u, fre, fim, fimn)                # V = F@U
                vt = ct["ft"]
                ptrans(vt, v)                                # V^T
                w = ct["w"]
                mmstage(w, vt, fre, fim, fimn, mask_t=m2t)   # W^T = m2t*(F@V^T)
                rp = ct["fr"]
                mmstage(rp, w, fre, fimn, fim)               # R' = Fi@W^T
                rpt = ct["ft"]
                ptrans(rpt, rp)                              # R'^T
                for m in range(3):
                    ps_re = psA.tile([128, 384], F32, tag="psr", name="psr")
                    ps_im = psA.tile([128, 384], F32, tag="psi", name="psi")
                    cmm(ps_re, ps_im, fre, fimn, fim, rpt, m)   # Z = Fi@R'^T
                    tm = kpool.tile([128, 384], F32, tag="tm", name="tm")
                    combine(accvec, ct["s"], ps_re, ps_im, m, tm)

            def dot2(dst, a0, a1, b0, b1, op_out):
                nc.vector.scalar_tensor_tensor(
                    junk[:, :, :], a0[:, :, :], 1.0, b0[:, :, :],
                    op0=MULT, op1=MULT, accum_out=sc["t0"][:, :])
                nc.vector.scalar_tensor_tensor(
                    junk[:, :, :], a1[:, :, :], 1.0, b1[:, :, :],
                    op0=MULT, op1=MULT, accum_out=sc["t1"][:, :])
                nc.vector.tensor_tensor(dst[:, :], sc["t0"][:, :], sc["t1"][:, :],
                                        op=op_out)

            def allred(t):
                ps = psB.tile([128, 1], F32, tag="pst", name="psred")
                nc.tensor.matmul(ps[:, :], ones[:, :], t[:, :], start=True, stop=True)
                nc.scalar.copy(t[:, :], ps[:, :])

            # ---- rhs = AH(y) + lam*x  (into r) ----
            for comp in acc + r + xs:
                nc.vector.memset(comp[:, :, :], 0.0)

            for c in range(C):
                ct = coil_tiles()
                load_coil(c, ct["s"], y_re, y_im, ct["sb"])
                w = ct["w"]
                nc.vector.tensor_tensor(w[0][:, :, :], ct["s"][0][:, :, :],
                                        m2[:, :, :], op=MULT)
                nc.vector.tensor_tensor(w[1][:, :, :], ct["s"][1][:, :, :],
                                        m2[:, :, :], op=MULT)
                pm = ct["fr"]
                mmstage(pm, w, fre, fimn, fim)         # P = Fi@W  [h, kw]
                pt = ct["ft"]
                ptrans(pt, pm)                         # P^T [kw, h]
                zt = (ct["u2"][0].bitcast(F32), ct["u2"][1].bitcast(F32))
                mmstage((ct["u2"][0], ct["u2"][1]), pt, fre, fimn, fim)  # Z^T
                zf = ct["fr"]
                nc.vector.tensor_copy(zf[0][:, :, :], zt[0][:, :, :])
                nc.vector.tensor_copy(zf[1][:, :, :], zt[1][:, :, :])
                load_coil(c, ct["s"], s_re, s_im, ct["sb"])
                for m in range(3):
                    ps_re = psB.tile([128, 384], F32, tag="pst", name="pst")
                    ps_im = psA.tile([128, 384], F32, tag="psr", name="psr")
                    for jp in range(3):
                        nc.tensor.matmul(ps_re[:, jp * 128:(jp + 1) * 128],
                                         zf[0][:, jp, m * 128:(m + 1) * 128],
                                         ident[:, :], is_transpose=True)
                        nc.tensor.matmul(ps_im[:, jp * 128:(jp + 1) * 128],
                                         zf[1][:, jp, m * 128:(m + 1) * 128],
                                         ident[:, :], is_transpose=True)
                    tm = kpool.tile([128, 384], F32, tag="tm", name="tm")
                    combine(r, ct["s"], ps_re, ps_im, m, tm)

            xf = (kpool.tile([128, 3, 384], F32, tag="sre", name="sre"),
                  kpool.tile([128, 3, 384], F32, tag="sim", name="sim"))
            nc.sync.dma_start(xf[0][:, :, :], r3(x_re))
            nc.sync.dma_start(xf[1][:, :, :], r3(x_im))
            nc.vector.scalar_tensor_tensor(r[0][:, :, :], xf[0][:, :, :],
                                           lam_t[:, 0:1], r[0][:, :, :],
                                           op0=MULT, op1=ADD)
            nc.vector.scalar_tensor_tensor(r[1][:, :, :], xf[1][:, :, :],
                                           lam_t[:, 0:1], r[1][:, :, :],
                                           op0=MULT, op1=ADD)
            nc.vector.tensor_copy(p[0][:, :, :], r[0][:, :, :])
            nc.vector.tensor_copy(p[1][:, :, :], r[1][:, :, :])
            dot2(sc["rtr"], r[0], r[1], r[0], r[1], ADD)
            allred(sc["rtr"])

            # ---- CG iterations ----
            def iteration(_=None):
                nc.vector.tensor_scalar(acc[0][:, :, :], p[0][:, :, :],
                                        lam_t[:, 0:1], None, op0=MULT)
                nc.vector.tensor_scalar(acc[1][:, :, :], p[1][:, :, :],
                                        lam_t[:, 0:1], None, op0=MULT)
                for c in range(C):
                    ct = coil_tiles()
                    load_coil(c, ct["s"], s_re, s_im, ct["sb"])
                    coil_chain(ct, p, acc)
                # pq = <p, q>
                dot2(sc["pqr"], p[0], p[1], acc[0], acc[1], ADD)
                allred(sc["pqr"])
                nc.vector.scalar_tensor_tensor(
                    junk[:, :, :], p[0][:, :, :], 1.0, acc[1][:, :, :],
                    op0=MULT, op1=MULT, accum_out=sc["t0"][:, :])
                nc.vector.scalar_tensor_tensor(
                    junk[:, :, :], p[1][:, :, :], 1.0, acc[0][:, :, :],
                    op0=MULT, op1=MULT, accum_out=sc["t1"][:, :])
                nc.vector.tensor_tensor(sc["pqi"][:, :], sc["t0"][:, :],
                                        sc["t1"][:, :], op=SUB)
                allred(sc["pqi"])
                # alpha = rtr*(pqr - i pqi)/(pqr^2+pqi^2)
                nc.vector.tensor_tensor(sc["t0"][:, :], sc["pqr"][:, :], sc["pqr"][:, :], op=MULT)
                nc.vector.tensor_tensor(sc["t1"][:, :], sc["pqi"][:, :], sc["pqi"][:, :], op=MULT)
                nc.vector.tensor_tensor(sc["t0"][:, :], sc["t0"][:, :], sc["t1"][:, :], op=ADD)
                nc.vector.reciprocal(sc["t0"][:, :], sc["t0"][:, :])
                nc.vector.tensor_tensor(sc["t1"][:, :], sc["rtr"][:, :], sc["t0"][:, :], op=MULT)
                nc.vector.tensor_tensor(sc["alr"][:, :], sc["pqr"][:, :], sc["t1"][:, :], op=MULT)
                nc.vector.tensor_tensor(sc["nali"][:, :], sc["pqi"][:, :], sc["t1"][:, :], op=MULT)
                nc.vector.tensor_scalar_mul(sc["ali"][:, :], sc["nali"][:, :], -1.0)
                nc.vector.tensor_scalar_mul(sc["nalr"][:, :], sc["alr"][:, :], -1.0)
                # x += alpha*p ; r -= alpha*q
                nc.vector.scalar_tensor_tensor(xs[0][:, :, :], p[0][:, :, :], sc["alr"][:, 0:1],
                                               xs[0][:, :, :], op0=MULT, op1=ADD)
                nc.vector.scalar_tensor_tensor(xs[0][:, :, :], p[1][:, :, :], sc["nali"][:, 0:1],
                                               xs[0][:, :, :], op0=MULT, op1=ADD)
                nc.vector.scalar_tensor_tensor(xs[1][:, :, :], p[1][:, :, :], sc["alr"][:, 0:1],
                                               xs[1][:, :, :], op0=MULT, op1=ADD)
                nc.vector.scalar_tensor_tensor(xs[1][:, :, :], p[0][:, :, :], sc["ali"][:, 0:1],
                                               xs[1][:, :, :], op0=MULT, op1=ADD)
                nc.vector.scalar_tensor_tensor(r[0][:, :, :], acc[0][:, :, :], sc["nalr"][:, 0:1],
                                               r[0][:, :, :], op0=MULT, op1=ADD)
                nc.vector.scalar_tensor_tensor(r[0][:, :, :], acc[1][:, :, :], sc["ali"][:, 0:1],
                                               r[0][:, :, :], op0=MULT, op1=ADD)
                nc.vector.scalar_tensor_tensor(r[1][:, :, :], acc[1][:, :, :], sc["nalr"][:, 0:1],
                                               r[1][:, :, :], op0=MULT, op1=ADD)
                nc.vector.scalar_tensor_tensor(r[1][:, :, :], acc[0][:, :, :], sc["nali"][:, 0:1],
                                               r[1][:, :, :], op0=MULT, op1=ADD)
                # rtrn, beta, p update
                dot2(sc["rtrn"], r[0], r[1], r[0], r[1], ADD)
                allred(sc["rtrn"])
                nc.vector.reciprocal(sc["t0"][:, :], sc["rtr"][:, :])
                nc.vector.tensor_tensor(sc["bet"][:, :], sc["rtrn"][:, :], sc["t0"][:, :], op=MULT)
                nc.vector.scalar_tensor_tensor(p[0][:, :, :], p[0][:, :, :], sc["bet"][:, 0:1],
                                               r[0][:, :, :], op0=MULT, op1=ADD)
                nc.vector.scalar_tensor_tensor(p[1][:, :, :], p[1][:, :, :], sc["bet"][:, 0:1],
                                               r[1][:, :, :], op0=MULT, op1=ADD)
                nc.vector.tensor_copy(sc["rtr"][:, :], sc["rtrn"][:, :])

            with tc.For_i(0, _ITERS) as _:
                iteration()

            nc.sync.dma_start(o_re.rearrange("(j p) w -> p j w", p=128), xs[0][:, :, :])
            nc.sync.dma_start(o_im.rearrange("(j p) w -> p j w", p=128), xs[1][:, :, :])

    nc.compile()
    return nc


class _Runner:
    """Build-once/run-many wrapper over bass2jax on n_cores axon TRN2 cores."""

    def __init__(self, nc, n_cores):
        import jax
        from jax.sharding import Mesh, PartitionSpec
        from jax.experimental.shard_map import shard_map
        from concourse.bass2jax import (_bass_exec_p, install_neuronx_cc_hook,
                                        partition_id_tensor)
        install_neuronx_cc_hook()
        self.n_cores = n_cores
        partition_name = nc.partition_id_tensor.name if nc.partition_id_tensor else None
        in_names, out_names, out_avals, zero_outs = [], [], [], []
        for alloc in nc.m.functions[0].allocations:
            if not isinstance(alloc, mybir.MemoryLocationSet):
                continue
            name = alloc.memorylocations[0].name
            if alloc.kind == "ExternalInput":
                if name != partition_name:
                    in_names.append(name)
            elif alloc.kind == "ExternalOutput":
                shape = tuple(alloc.tensor_shape)
                dtype = mybir.dt.np(alloc.dtype)
                out_names.append(name)
                out_avals.append(jax.core.ShapedArray(shape, dtype))
                zero_outs.append(np.zeros(shape, dtype))
        self.in_names, self.out_names = in_names, out_names
        self.out_avals, self.zero_outs = out_avals, zero_outs
        n_params, n_outs = len(in_names), len(out_avals)
        all_in = list(in_names) + list(out_names)
        if partition_name is not None:
            all_in.append(partition_name)

        def _body(*args):
            operands = list(args)
            if partition_name is not None:
                operands.append(partition_id_tensor())
            outs = _bass_exec_p.bind(
                *operands, out_avals=tuple(out_avals), in_names=tuple(all_in),
                out_names=tuple(out_names), lowering_input_output_aliases=(),
                sim_require_finite=True, sim_require_nnan=True, nc=nc)
            return tuple(outs)

        donate = tuple(range(n_params, n_params + n_outs))
        devices = jax.devices()[:n_cores]
        mesh = Mesh(np.asarray(devices), ("core",))
        in_specs = (PartitionSpec("core"),) * (n_params + n_outs)
        out_specs = (PartitionSpec("core"),) * len(out_names)
        self._fn = jax.jit(
            shard_map(_body, mesh=mesh, in_specs=in_specs, out_specs=out_specs,
                      check_rep=False),
            donate_argnums=donate, keep_unused=True)
        self.n_params = n_params

    def run(self, in_maps):
        per_core = [[np.asarray(m[n]) for n in self.in_names] for m in in_maps]
        concat_in = [np.concatenate([per_core[c][i] for c in range(self.n_cores)],
                                    axis=0) for i in range(self.n_params)]
        concat_zeros = [np.zeros((self.n_cores * z.shape[0], *z.shape[1:]), z.dtype)
                        for z in self.zero_outs]
        out_arrs = self._fn(*concat_in, *concat_zeros)
        return [
            {n: np.asarray(out_arrs[i]).reshape(self.n_cores,
                                                *self.out_avals[i].shape)[c]
             for i, n in enumerate(self.out_names)}
            for c in range(self.n_cores)
        ]


_RUNNER = None


def _get_runner():
    global _RUNNER
    if _RUNNER is None:
        _RUNNER = _Runner(_build_nc(), _B)
    return _RUNNER


def _make_consts():
    N = _N
    k = np.arange(N)
    ang = -2.0 * np.pi * np.outer(k, k) / N
    fre = (np.cos(ang) / np.sqrt(N)).astype(np.float32)
    fim = (np.sin(ang) / np.sqrt(N)).astype(np.float32)
    return fre, fim, (-fim).astype(np.float32)


def kernel(lambdaa, x, y, smaps, mask, mu):
    lambdaa = np.asarray(lambdaa, np.float32)
    x = np.asarray(x, np.float32)
    y = np.asarray(y, np.float32)
    smaps = np.asarray(smaps, np.float32)
    fre, fim, fimn = _make_consts()
    m = np.asarray(mask).astype(np.float32)
    mt = np.ascontiguousarray(m.T)
    lam_b = np.full((128, 1), float(lambdaa[0]), np.float32)
    runner = _get_runner()
    in_maps = []
    for b in range(_B):
        in_maps.append({
            "s_re": np.ascontiguousarray(smaps[b, :, :, :, 0]).astype(_BF16NP),
            "s_im": np.ascontiguousarray(smaps[b, :, :, :, 1]).astype(_BF16NP),
            "y_re": np.ascontiguousarray(y[b, :, :, :, 0]).astype(_BF16NP),
            "y_im": np.ascontiguousarray(y[b, :, :, :, 1]).astype(_BF16NP),
            "x_re": np.ascontiguousarray(x[b, :, :, 0]),
            "x_im": np.ascontiguousarray(x[b, :, :, 1]),
            "m2": m, "m2t": mt,
            "fre": fre, "fim": fim, "fimn": fimn,
            "lam_b": lam_b,
        })
    outs = runner.run(in_maps)
    result = np.empty((_B, _N, _N, 2), np.float32)
    for b in range(_B):
        result[b, :, :, 0] = outs[b]["o_re"]
        result[b, :, :, 1] = outs[b]["o_im"]
    return result
